# revision 8
# baseline (speedup 1.0000x reference)
"""GAT attention block kernel for Trainium2 (8 NeuronCores, data-parallel over B).

Math: for the reference
    sq_proj = ai_sq @ W^T + b;  sn_proj[n] = ai_sn[n] @ W^T + b
    mult[n,b] = u1.sq_proj[b] + u2.sn_proj[n,b]
    out = softmax-ish over n of exp(leaky_relu(mult))
which algebraically reduces (v1 = W^T u1, v2 = W^T u2, c = (u1+u2).b) to
    mult[n,b] = ai_sq[b].v1 + ai_sn[n,b].v2 + c
so the only heavy work is a streaming per-row dot product of ai_sn with v2
(memory-bound: 128 MB of ai_sn traffic).

Sharding: B axis split across the 8 cores (1024 nodes each); W/b/u folded on
host into v1/v2/c and replicated.

Device layout per core:
  rows r = n*1024 + b_local, grouped g = r // 128 = n*8 + j  (j = b_local//128,
  p = b_local%128 on partitions).  T[p, g] = row-dot for (n, j, p).
  The dot products are computed by a custom fused DVE op (running cumsum of
  x*v2 over the free stream) + strided diffs, one 1x pass instead of
  mul+reduce two passes.  Epilogue (leaky-relu/exp/normalize over n) is tiny.
  Output [32,1024] needs (n,j) on partitions -> two 128x128 PE transposes.

`reps` repeats the whole computation inside one NEFF (benchmarking only:
wall-clock slope over reps isolates exec time from the ~100ms dispatch).
"""

import numpy as np

N_CORES = 8
N_NEIGH = 32
BATCH = 8192
IN_DIM = 128
B_SH = BATCH // N_CORES          # 1024 nodes per core
NROWS = N_NEIGH * B_SH           # 32768 rows of ai_sn per core
G_TOT = NROWS // 128             # 256 row-groups (columns of T), g = n*8+j
GQ = B_SH // 128                 # 8 groups for the ai_sq matvec
GPC = 32                         # row-groups per DMA chunk (2 MB chunks)
N_CHUNKS = G_TOT // GPC          # 8
J = B_SH // 128                  # 8
MODE = "scan"                    # "scan" (fused custom DVE op) or "2pass"
LRELU_ON_ACT = True              # leaky-relu via ACT Lrelu(alpha) vs DVE max

_CACHE = {}


def _register_scan_op():
    """Register the fused multiply-cumsum DVE op (out[k] = sum_{i<=k} in0*in1)."""
    from concourse import dve_ops
    from concourse.dve_spec import Spec, Src0, Src1, scan, AluOp, lower, _has_src1
    from concourse.dve_uop import DveOpSpec

    NAME = "MUL_CUMSUM_GAT_ANT"
    for op in dve_ops.OPS:
        if op.name == NAME:
            return op

    def _ref(in0, in1):
        prod = (np.asarray(in0, np.float32) * np.asarray(in1, np.float32))
        flat = prod.reshape(prod.shape[0], -1)
        return np.cumsum(flat, axis=-1, dtype=np.float32).reshape(prod.shape)

    spec = Spec(body=scan(AluOp.ADD, Src0 * Src1), reference=_ref)
    row = max(dve_ops._SUB_OPCODE_FOR_NAME.values()) + 1
    assert row < 0x20
    shas = {}
    for ver in ("v3", "v4"):
        uops = lower(spec, ver=ver)
        shas[ver] = DveOpSpec(
            name=NAME, opcode=row, uops=uops, rd1_en=_has_src1(spec)
        ).sha(ver)
    op = dve_ops.DveOp(NAME, spec, subdim=False, uops_sha=shas)
    dve_ops.OPS.append(op)
    dve_ops.CUSTOM_DVE_SPECS[NAME] = spec
    dve_ops._SUB_OPCODE_FOR_NAME[NAME] = row
    return op


def _reap(tile_ap, dims, offset=0):
    """View a tile under custom free-dim APs ([step, count] pairs, elements)."""
    import concourse.bass as bass

    a = tile_ap[:] if not isinstance(tile_ap, bass.AP) else tile_ap
    return bass.AP(
        tensor=a.tensor, offset=a.offset + offset,
        ap=[list(a.ap[0])] + [list(d) for d in dims],
    )


def _build(mode=MODE, reps=1):
    import concourse.bacc as bacc
    import concourse.bass as bass
    import concourse.tile as tile
    import concourse.mybir as mybir
    from concourse.masks import make_identity

    f32 = mybir.dt.float32
    scan_op = _register_scan_op() if mode == "scan" else None

    nc = bacc.Bacc("TRN2", target_bir_lowering=False, debug=False,
                   num_devices=N_CORES)

    x = nc.dram_tensor("x", [NROWS, IN_DIM], f32, kind="ExternalInput")
    xq = nc.dram_tensor("xq", [B_SH, IN_DIM], f32, kind="ExternalInput")
    # prm columns: 0:128 = v2 (bcast to all partitions), 128:256 = v1, 256 = c
    prm = nc.dram_tensor("prm", [128, 258], f32, kind="ExternalInput")
    y = nc.dram_tensor("y", [N_NEIGH, B_SH], f32, kind="ExternalOutput")

    x_r = x.rearrange("(g p) f -> p g f", p=128)     # [128, 256, 128]
    xq_r = xq.rearrange("(g p) f -> p g f", p=128)   # [128, 8, 128]

    with tile.TileContext(nc) as tc:
        with (
            tc.tile_pool(name="consts", bufs=1) as consts,
            tc.tile_pool(name="xin", bufs=4) as xin,
            tc.tile_pool(name="mid", bufs=2) as mid,
            tc.tile_pool(name="acc", bufs=2) as acc,
            tc.tile_pool(name="ep", bufs=2) as ep,
            tc.tile_pool(name="psum", bufs=2, space="PSUM") as psum,
            tc.tile_pool(name="outp", bufs=2) as outp,
        ):
            prm_t = consts.tile([128, 258], f32)
            nc.sync.dma_start(out=prm_t, in_=prm[:, :])
            v2b = prm_t[:, 0:128]
            v1b = prm_t[:, 128:256]
            cvec = prm_t[:, 256:257]

            ident = consts.tile([128, 128], f32)
            make_identity(nc, ident)

            # Replicate v2 across GPC groups and v1 across GQ groups by doubling
            v2rep = consts.tile([128, GPC, 128], f32)
            nc.vector.tensor_copy(out=v2rep[:, 0:1, :], in_=v2b)
            d = 1
            while d < GPC:
                step = min(d, GPC - d)
                nc.vector.tensor_copy(
                    out=v2rep[:, d : d + step, :], in_=v2rep[:, 0:step, :]
                )
                d += step
            v1rep = consts.tile([128, GQ, 128], f32)
            nc.vector.tensor_copy(out=v1rep[:, 0:1, :], in_=v1b)
            d = 1
            while d < GQ:
                step = min(d, GQ - d)
                nc.vector.tensor_copy(
                    out=v1rep[:, d : d + step, :], in_=v1rep[:, 0:step, :]
                )
                d += step

            for _rep in range(reps):
                # T columns: 0..255 row-dots (g = n*8+j), 256..263 s[j] dots
                T = acc.tile([128, G_TOT + GQ], f32, tag="T")
                if mode == "scan":
                    Ends = acc.tile([128, G_TOT + GQ], f32, tag="Ends")
                else:
                    Ends = None

                # xq first (small, warms the pipe during the first big chunk)
                chunks = [(xq_r[:, :, :], v1rep, GQ, G_TOT)]
                for ci in range(N_CHUNKS):
                    chunks.append(
                        (x_r[:, ci * GPC : (ci + 1) * GPC, :], v2rep, GPC,
                         ci * GPC)
                    )

                for dram_ap, vrep, g, c0 in chunks:
                    xt = xin.tile([128, g, 128], f32, tag="xt")
                    nc.sync.dma_start(out=xt, in_=dram_ap)
                    if mode == "dmaonly":
                        continue
                    if mode == "scan":
                        yt = mid.tile([128, g, 128], f32, tag="yt")
                        nc.vector._custom_dve(
                            scan_op, out=yt[:, :, :], in0=xt[:, :, :],
                            in1=vrep[:, 0:g, :],
                        )
                        # per-group scan ends -> Ends columns
                        nc.vector.tensor_copy(
                            out=Ends[:, c0 : c0 + g], in_=yt[:, :, 127:128]
                        )
                    else:
                        yt = mid.tile([128, g, 128], f32, tag="yt")
                        nc.vector.tensor_mul(yt, xt, vrep[:, 0:g, :])
                        nc.vector.reduce_sum(
                            out=T[:, c0 : c0 + g], in_=yt[:, :, :],
                            axis=mybir.AxisListType.X,
                        )

                if mode == "dmaonly":
                    # floor measurement: skip all compute, emit dummy output
                    y_r0 = y.rearrange("(h n) (j p) -> h n j p", h=2, p=128)
                    for h in range(2):
                        ot = outp.tile([128, 128], f32, tag="ot")
                        nc.vector.memset(ot[:, :], 0.0)
                        nc.sync.dma_start(out=y_r0[h], in_=ot[:, :])
                    continue

                if mode == "scan":
                    # T[g] = Ends[g] - Ends[g-1], except scan-restart
                    # boundaries (cols 0,32,..,224,256) where T[g] = Ends[g].
                    nc.vector.tensor_sub(
                        T[:, 1 : G_TOT + GQ], Ends[:, 1 : G_TOT + GQ],
                        Ends[:, 0 : G_TOT + GQ - 1],
                    )
                    nc.vector.tensor_copy(
                        out=_reap(T, [[GPC, 9]]), in_=_reap(Ends, [[GPC, 9]])
                    )

                # s[j] += c
                nc.vector.tensor_scalar_add(
                    T[:, G_TOT : G_TOT + GQ], T[:, G_TOT : G_TOT + GQ], cvec
                )

                # broadcast s over n: S256[p, n, j] = s[p, j]
                S256 = ep.tile([128, N_NEIGH, J], f32, tag="S256")
                nc.vector.tensor_copy(
                    out=S256[:, 0:1, :], in_=T[:, G_TOT : G_TOT + GQ]
                )
                d = 1
                while d < N_NEIGH:
                    step = min(d, N_NEIGH - d)
                    nc.vector.tensor_copy(
                        out=S256[:, d : d + step, :], in_=S256[:, 0:step, :]
                    )
                    d += step

                # mult = t + s ; leaky-relu ; exp
                M = ep.tile([128, N_NEIGH, J], f32, tag="M")
                T3 = _reap(T, [[J, N_NEIGH], [1, J]])
                nc.vector.tensor_add(M, T3, S256)
                L = ep.tile([128, N_NEIGH, J], f32, tag="L")
                M2 = _reap(M, [[1, N_NEIGH * J]])
                L2 = _reap(L, [[1, N_NEIGH * J]])
                if LRELU_ON_ACT:
                    nc.scalar.activation(
                        out=L2, in_=M2,
                        func=mybir.ActivationFunctionType.Lrelu, alpha=0.01,
                    )
                else:
                    nc.vector.tensor_scalar_mul(L, M, 0.01)
                    nc.vector.tensor_max(L, L, M)
                E = ep.tile([128, N_NEIGH, J], f32, tag="E")
                E2 = _reap(E, [[1, N_NEIGH * J]])
                nc.scalar.activation(
                    out=E2, in_=L2, func=mybir.ActivationFunctionType.Exp
                )

                # denom over n, reciprocal, broadcast, normalize
                D = ep.tile([128, J], f32, tag="D")
                nc.vector.reduce_sum(
                    out=D, in_=_reap(E, [[1, J], [J, N_NEIGH]]),
                    axis=mybir.AxisListType.X,
                )
                R = ep.tile([128, J], f32, tag="R")
                nc.vector.reciprocal(R, D)
                R256 = ep.tile([128, N_NEIGH, J], f32, tag="R256")
                nc.vector.tensor_copy(out=R256[:, 0:1, :], in_=R)
                d = 1
                while d < N_NEIGH:
                    step = min(d, N_NEIGH - d)
                    nc.vector.tensor_copy(
                        out=R256[:, d : d + step, :], in_=R256[:, 0:step, :]
                    )
                    d += step
                O = ep.tile([128, N_NEIGH, J], f32, tag="O")
                nc.vector.tensor_mul(O, E, R256)

                # transpose [p, c] -> [c, p] in two 128x128 halves, DMA out
                y_r = y.rearrange("(h n) (j p) -> h n j p", h=2, p=128)
                for h in range(2):
                    pt = psum.tile([128, 128], f32, tag="pt")
                    nc.tensor.transpose(
                        pt[:, :], _reap(O, [[1, 128]], offset=h * 128),
                        ident[:, :],
                    )
                    ot = outp.tile([128, 128], f32, tag="ot")
                    nc.scalar.copy(ot[:, :], pt[:, :])
                    nc.sync.dma_start(out=y_r[h], in_=ot[:, :])

    nc.finalize()
    return nc


def _get_nc(mode=MODE, reps=1):
    key = ("nc", mode, reps)
    if key not in _CACHE:
        _CACHE[key] = _build(mode, reps)
    return _CACHE[key]


def kernel(ai_sq, ai_sn, W_w, W_b, u, _trace=False, _mode=None, _reps=1):
    from concourse.bass_utils import run_bass_kernel_spmd

    mode = _mode or MODE
    ai_sq = np.ascontiguousarray(np.asarray(ai_sq, dtype=np.float32))
    ai_sn = np.ascontiguousarray(np.asarray(ai_sn, dtype=np.float32))
    W_w = np.asarray(W_w, dtype=np.float32)
    W_b = np.asarray(W_b, dtype=np.float32)
    u = np.asarray(u, dtype=np.float32)

    out_dim = W_w.shape[0]
    u1, u2 = u[:out_dim], u[out_dim:]
    v1 = W_w.T @ u1
    v2 = W_w.T @ u2
    c = float((u1 + u2) @ W_b)
    prm = np.zeros((128, 258), np.float32)
    prm[:, 0:128] = v2[None, :]
    prm[:, 128:256] = v1[None, :]
    prm[:, 256] = c

    in_maps = []
    for k in range(N_CORES):
        sl = slice(k * B_SH, (k + 1) * B_SH)
        in_maps.append({
            "x": np.ascontiguousarray(ai_sn[:, sl, :]).reshape(NROWS, IN_DIM),
            "xq": np.ascontiguousarray(ai_sq[sl, :]),
            "prm": prm,
        })

    nc = _get_nc(mode, _reps)
    res = run_bass_kernel_spmd(
        nc, in_maps, core_ids=list(range(N_CORES)), trace=_trace
    )
    _CACHE["last_results"] = res
    return np.concatenate([r["y"] for r in res.results], axis=1)


# revision 12
# speedup vs baseline: 1.4269x; 1.4269x over previous
"""GAT attention block kernel for Trainium2 (8 NeuronCores, data-parallel over B).

Math: for the reference
    sq_proj = ai_sq @ W^T + b;  sn_proj[n] = ai_sn[n] @ W^T + b
    mult[n,b] = u1.sq_proj[b] + u2.sn_proj[n,b]
    out = softmax-ish over n of exp(leaky_relu(mult))
which algebraically reduces (v1 = W^T u1, v2 = W^T u2, c = (u1+u2).b) to
    mult[n,b] = ai_sq[b].v1 + ai_sn[n,b].v2 + c
so the only heavy work is a streaming per-row dot product of ai_sn with v2
(memory-bound: 128 MB of ai_sn traffic).

Sharding: B axis split across the 8 cores (1024 nodes each); W/b/u folded on
host into v1/v2/c and replicated.

Device layout per core:
  rows r = n*1024 + b_local, grouped g = r // 128 = n*8 + j  (j = b_local//128,
  p = b_local%128 on partitions).  T[p, g] = row-dot for (n, j, p).
  The dot products are computed by a custom fused DVE op (running cumsum of
  x*v2 over the free stream) + strided diffs, one 1x pass instead of
  mul+reduce two passes.  Epilogue (leaky-relu/exp/normalize over n) is tiny.
  Output [32,1024] needs (n,j) on partitions -> two 128x128 PE transposes.

`reps` repeats the whole computation inside one NEFF (benchmarking only:
wall-clock slope over reps isolates exec time from the ~100ms dispatch).
"""

import numpy as np

N_CORES = 8
N_NEIGH = 32
BATCH = 8192
IN_DIM = 128
B_SH = BATCH // N_CORES          # 1024 nodes per core
NROWS = N_NEIGH * B_SH           # 32768 rows of ai_sn per core
G_TOT = NROWS // 128             # 256 row-groups (columns of T), g = n*8+j
GQ = B_SH // 128                 # 8 groups for the ai_sq matvec
GPC = 32                         # row-groups per DMA chunk (2 MB chunks)
N_CHUNKS = G_TOT // GPC          # 8
J = B_SH // 128                  # 8
MODE = "scan"                    # "scan" (fused custom DVE op) or "2pass"
LRELU_ON_ACT = True              # leaky-relu via ACT Lrelu(alpha) vs DVE max

_CACHE = {}


def _register_scan_op():
    """Register the fused multiply-cumsum DVE op (out[k] = sum_{i<=k} in0*in1)."""
    from concourse import dve_ops
    from concourse.dve_spec import Spec, Src0, Src1, scan, AluOp, lower, _has_src1
    from concourse.dve_uop import DveOpSpec

    NAME = "MUL_CUMSUM_GAT_ANT"
    for op in dve_ops.OPS:
        if op.name == NAME:
            return op

    def _ref(in0, in1):
        prod = (np.asarray(in0, np.float32) * np.asarray(in1, np.float32))
        flat = prod.reshape(prod.shape[0], -1)
        return np.cumsum(flat, axis=-1, dtype=np.float32).reshape(prod.shape)

    spec = Spec(body=scan(AluOp.ADD, Src0 * Src1), reference=_ref)
    row = max(dve_ops._SUB_OPCODE_FOR_NAME.values()) + 1
    assert row < 0x20
    shas = {}
    for ver in ("v3", "v4"):
        uops = lower(spec, ver=ver)
        shas[ver] = DveOpSpec(
            name=NAME, opcode=row, uops=uops, rd1_en=_has_src1(spec)
        ).sha(ver)
    op = dve_ops.DveOp(NAME, spec, subdim=False, uops_sha=shas)
    dve_ops.OPS.append(op)
    dve_ops.CUSTOM_DVE_SPECS[NAME] = spec
    dve_ops._SUB_OPCODE_FOR_NAME[NAME] = row
    return op


def _reap(tile_ap, dims, offset=0):
    """View a tile under custom free-dim APs ([step, count] pairs, elements)."""
    import concourse.bass as bass

    a = tile_ap[:] if not isinstance(tile_ap, bass.AP) else tile_ap
    return bass.AP(
        tensor=a.tensor, offset=a.offset + offset,
        ap=[list(a.ap[0])] + [list(d) for d in dims],
    )


def _build(mode=MODE, reps=1, gpc=GPC, dual_ring=False, xin_bufs=4):
    import concourse.bacc as bacc
    import concourse.bass as bass
    import concourse.tile as tile
    import concourse.mybir as mybir
    from concourse.masks import make_identity

    f32 = mybir.dt.float32
    scan_op = _register_scan_op() if mode in ("scan", "dmaonly") else None
    n_chunks = G_TOT // gpc
    assert gpc % GPC == 0 or gpc == GPC

    nc = bacc.Bacc("TRN2", target_bir_lowering=False, debug=False,
                   num_devices=N_CORES)

    x = nc.dram_tensor("x", [NROWS, IN_DIM], f32, kind="ExternalInput")
    xq = nc.dram_tensor("xq", [B_SH, IN_DIM], f32, kind="ExternalInput")
    # prm columns: 0:128 = v2 (bcast to all partitions), 128:256 = v1, 256 = c
    prm = nc.dram_tensor("prm", [128, 258], f32, kind="ExternalInput")
    y = nc.dram_tensor("y", [N_NEIGH, B_SH], f32, kind="ExternalOutput")

    x_r = x.rearrange("(g p) f -> p g f", p=128)     # [128, 256, 128]
    xq_r = xq.rearrange("(g p) f -> p g f", p=128)   # [128, 8, 128]

    with tile.TileContext(nc) as tc:
        with (
            tc.tile_pool(name="consts", bufs=1) as consts,
            tc.tile_pool(name="xin", bufs=xin_bufs) as xin,
            tc.tile_pool(name="mid", bufs=3) as mid,
            tc.tile_pool(name="acc", bufs=2) as acc,
            tc.tile_pool(name="ep", bufs=2) as ep,
            tc.tile_pool(name="psum", bufs=2, space="PSUM") as psum,
            tc.tile_pool(name="outp", bufs=2) as outp,
        ):
            prm_t = consts.tile([128, 258], f32)
            nc.sync.dma_start(out=prm_t, in_=prm[:, :])
            v2b = prm_t[:, 0:128]
            v1b = prm_t[:, 128:256]
            cvec = prm_t[:, 256:257]

            ident = consts.tile([128, 128], f32)
            make_identity(nc, ident)

            # Replicate v2 across GPC groups and v1 across GQ groups by doubling
            v2rep = consts.tile([128, GPC, 128], f32)
            nc.vector.tensor_copy(out=v2rep[:, 0:1, :], in_=v2b)
            d = 1
            while d < GPC:
                step = min(d, GPC - d)
                nc.vector.tensor_copy(
                    out=v2rep[:, d : d + step, :], in_=v2rep[:, 0:step, :]
                )
                d += step
            v1rep = consts.tile([128, GQ, 128], f32)
            nc.vector.tensor_copy(out=v1rep[:, 0:1, :], in_=v1b)
            d = 1
            while d < GQ:
                step = min(d, GQ - d)
                nc.vector.tensor_copy(
                    out=v1rep[:, d : d + step, :], in_=v1rep[:, 0:step, :]
                )
                d += step

            for _rep in range(reps):
                # T columns: 0..255 row-dots (g = n*8+j), 256..263 s[j] dots
                T = acc.tile([128, G_TOT + GQ], f32, tag="T")
                if mode == "scan":
                    Ends = acc.tile([128, G_TOT + GQ], f32, tag="Ends")
                else:
                    Ends = None

                # xq first (small, warms the pipe during the first big chunk)
                chunks = [(xq_r[:, :, :], GQ, G_TOT)]
                for ci in range(n_chunks):
                    chunks.append(
                        (x_r[:, ci * gpc : (ci + 1) * gpc, :], gpc, ci * gpc)
                    )

                for di, (dram_ap, g, c0) in enumerate(chunks):
                    xt = xin.tile([128, g, 128], f32, tag="xt")
                    dma_eng = (
                        nc.scalar if (dual_ring and di % 2 == 1) else nc.sync
                    )
                    dma_eng.dma_start(out=xt, in_=dram_ap)
                    if mode == "dmaonly":
                        continue
                    # compute in sub-chunks of <= GPC groups (v2rep period)
                    for s0 in range(0, g, GPC):
                        sg = min(GPC, g - s0)
                        vrep = v1rep if g == GQ else v2rep
                        sc0 = c0 + s0
                        if mode == "scan":
                            yt = mid.tile([128, sg, 128], f32, tag="yt")
                            nc.vector._custom_dve(
                                scan_op, out=yt[:, :, :],
                                in0=xt[:, s0 : s0 + sg, :],
                                in1=vrep[:, 0:sg, :],
                            )
                            # per-group scan ends -> Ends columns
                            nc.vector.tensor_copy(
                                out=Ends[:, sc0 : sc0 + sg],
                                in_=yt[:, :, 127:128],
                            )
                        else:
                            yt = mid.tile([128, sg, 128], f32, tag="yt")
                            nc.vector.tensor_mul(
                                yt, xt[:, s0 : s0 + sg, :], vrep[:, 0:sg, :]
                            )
                            nc.vector.reduce_sum(
                                out=T[:, sc0 : sc0 + sg], in_=yt[:, :, :],
                                axis=mybir.AxisListType.X,
                            )

                if mode == "dmaonly":
                    # floor measurement: skip all compute, emit dummy output
                    y_r0 = y.rearrange("(h n) (j p) -> h n j p", h=2, p=128)
                    for h in range(2):
                        ot = outp.tile([128, 128], f32, tag="ot")
                        nc.vector.memset(ot[:, :], 0.0)
                        nc.sync.dma_start(out=y_r0[h], in_=ot[:, :])
                    continue

                if mode == "scan":
                    # T[g] = Ends[g] - Ends[g-1], except scan-restart
                    # boundaries (cols 0,32,..,224,256) where T[g] = Ends[g].
                    nc.vector.tensor_sub(
                        T[:, 1 : G_TOT + GQ], Ends[:, 1 : G_TOT + GQ],
                        Ends[:, 0 : G_TOT + GQ - 1],
                    )
                    nc.vector.tensor_copy(
                        out=_reap(T, [[GPC, 9]]), in_=_reap(Ends, [[GPC, 9]])
                    )

                # s[j] += c
                nc.vector.tensor_scalar_add(
                    T[:, G_TOT : G_TOT + GQ], T[:, G_TOT : G_TOT + GQ], cvec
                )

                # broadcast s over n: S256[p, n, j] = s[p, j]
                S256 = ep.tile([128, N_NEIGH, J], f32, tag="S256")
                nc.vector.tensor_copy(
                    out=S256[:, 0:1, :], in_=T[:, G_TOT : G_TOT + GQ]
                )
                d = 1
                while d < N_NEIGH:
                    step = min(d, N_NEIGH - d)
                    nc.vector.tensor_copy(
                        out=S256[:, d : d + step, :], in_=S256[:, 0:step, :]
                    )
                    d += step

                # mult = t + s ; leaky-relu ; exp
                M = ep.tile([128, N_NEIGH, J], f32, tag="M")
                T3 = _reap(T, [[J, N_NEIGH], [1, J]])
                nc.vector.tensor_add(M, T3, S256)
                L = ep.tile([128, N_NEIGH, J], f32, tag="L")
                M2 = _reap(M, [[1, N_NEIGH * J]])
                L2 = _reap(L, [[1, N_NEIGH * J]])
                if LRELU_ON_ACT:
                    nc.scalar.activation(
                        out=L2, in_=M2,
                        func=mybir.ActivationFunctionType.Lrelu, alpha=0.01,
                    )
                else:
                    nc.vector.tensor_scalar_mul(L, M, 0.01)
                    nc.vector.tensor_max(L, L, M)
                E = ep.tile([128, N_NEIGH, J], f32, tag="E")
                E2 = _reap(E, [[1, N_NEIGH * J]])
                nc.scalar.activation(
                    out=E2, in_=L2, func=mybir.ActivationFunctionType.Exp
                )

                # denom over n, reciprocal, broadcast, normalize
                D = ep.tile([128, J], f32, tag="D")
                nc.vector.reduce_sum(
                    out=D, in_=_reap(E, [[1, J], [J, N_NEIGH]]),
                    axis=mybir.AxisListType.X,
                )
                R = ep.tile([128, J], f32, tag="R")
                nc.vector.reciprocal(R, D)
                R256 = ep.tile([128, N_NEIGH, J], f32, tag="R256")
                nc.vector.tensor_copy(out=R256[:, 0:1, :], in_=R)
                d = 1
                while d < N_NEIGH:
                    step = min(d, N_NEIGH - d)
                    nc.vector.tensor_copy(
                        out=R256[:, d : d + step, :], in_=R256[:, 0:step, :]
                    )
                    d += step
                O = ep.tile([128, N_NEIGH, J], f32, tag="O")
                nc.vector.tensor_mul(O, E, R256)

                # transpose [p, c] -> [c, p] in two 128x128 halves, DMA out
                y_r = y.rearrange("(h n) (j p) -> h n j p", h=2, p=128)
                for h in range(2):
                    pt = psum.tile([128, 128], f32, tag="pt")
                    nc.tensor.transpose(
                        pt[:, :], _reap(O, [[1, 128]], offset=h * 128),
                        ident[:, :],
                    )
                    ot = outp.tile([128, 128], f32, tag="ot")
                    nc.scalar.copy(ot[:, :], pt[:, :])
                    nc.sync.dma_start(out=y_r[h], in_=ot[:, :])

    nc.finalize()
    return nc


BUILD_OPTS = {}  # overridable: gpc, dual_ring, xin_bufs


def _get_nc(mode=MODE, reps=1, **opts):
    kw = dict(BUILD_OPTS)
    kw.update(opts)
    key = ("nc", mode, reps, tuple(sorted(kw.items())))
    if key not in _CACHE:
        _CACHE[key] = _build(mode, reps, **kw)
    return _CACHE[key]


def kernel(ai_sq, ai_sn, W_w, W_b, u, _trace=False, _mode=None, _reps=1):
    from concourse.bass_utils import run_bass_kernel_spmd

    mode = _mode or MODE
    ai_sq = np.ascontiguousarray(np.asarray(ai_sq, dtype=np.float32))
    ai_sn = np.ascontiguousarray(np.asarray(ai_sn, dtype=np.float32))
    W_w = np.asarray(W_w, dtype=np.float32)
    W_b = np.asarray(W_b, dtype=np.float32)
    u = np.asarray(u, dtype=np.float32)

    out_dim = W_w.shape[0]
    u1, u2 = u[:out_dim], u[out_dim:]
    v1 = W_w.T @ u1
    v2 = W_w.T @ u2
    c = float((u1 + u2) @ W_b)
    prm = np.zeros((128, 258), np.float32)
    prm[:, 0:128] = v2[None, :]
    prm[:, 128:256] = v1[None, :]
    prm[:, 256] = c

    in_maps = []
    for k in range(N_CORES):
        sl = slice(k * B_SH, (k + 1) * B_SH)
        in_maps.append({
            "x": np.ascontiguousarray(ai_sn[:, sl, :]).reshape(NROWS, IN_DIM),
            "xq": np.ascontiguousarray(ai_sq[sl, :]),
            "prm": prm,
        })

    nc = _get_nc(mode, _reps)
    res = run_bass_kernel_spmd(
        nc, in_maps, core_ids=list(range(N_CORES)), trace=_trace
    )
    _CACHE["last_results"] = res
    return np.concatenate([r["y"] for r in res.results], axis=1)
